# revision 1
# baseline (speedup 1.0000x reference)
"""Self-contained TP-over-heads DeepseekAttention kernel for 8 TRN2 cores.

Sharding: tensor-parallel across heads (4 heads/core). Each core computes
Q/K/V projections for its heads (bf16 matmuls), RoPE, attention with
transposed-scores layout (fp32r matmuls), a row-parallel partial o_proj
fused into the attention loop, then chunked ReduceScatter(add) over the
sequence dim. Host reassembles the 8 per-core [256, 4096] slices.
"""

import numpy as np
import ml_dtypes

import concourse.bass as bass
import concourse.mybir as mybir
import concourse.tile as tile
from concourse import bacc
from concourse.bass_utils import run_bass_kernel_spmd

# problem shapes (hardcoded per contract)
S = 2048
H = 4096
NH = 32
D = 128
NC = 8
HPC = NH // NC          # 4 heads per core
DPC = HPC * D           # 512 head-dims per core
KT = H // 128           # 32 contraction tiles over hidden
SCH = 512               # s-chunk for projections
NSC = S // SCH          # 4
ST = S // 128           # 16 s-tiles
QCH = 512               # q-chunk in attention
NQC = S // QCH          # 4
NKT = S // 128          # 16 k-tiles in attention
SPC = S // NC           # 256 rows of output per core
RS_CHUNKS = 4           # chunked ReduceScatter count (1 or NQC)

f32 = mybir.dt.float32
f32r = mybir.dt.float32r
bf16 = mybir.dt.bfloat16
bf16_np = ml_dtypes.bfloat16

ROPE_THETA = 10000.0
SCALE = float(1.0 / np.sqrt(D))

_CACHE: dict = {}


def _build(with_collective=True):
    from contextlib import ExitStack

    nc = bacc.Bacc("TRN2", target_bir_lowering=False, debug=False, num_devices=NC)

    # ---- I/O ----
    xt = nc.dram_tensor("xt", [KT, 128, S], bf16, kind="ExternalInput").ap()
    wq = nc.dram_tensor("wq", [KT, 128, DPC], bf16, kind="ExternalInput").ap()
    wk = nc.dram_tensor("wk", [KT, 128, DPC], bf16, kind="ExternalInput").ap()
    wv = nc.dram_tensor("wv", [KT, 128, DPC], bf16, kind="ExternalInput").ap()
    wo = nc.dram_tensor("wo", [HPC, 128, H], f32r, kind="ExternalInput").ap()
    cost = nc.dram_tensor("cost", [128, S], f32, kind="ExternalInput").ap()
    sint = nc.dram_tensor("sint", [128, S], f32, kind="ExternalInput").ap()
    rmat = nc.dram_tensor("rmat", [128, 128], f32r, kind="ExternalInput").ap()
    ones_col = nc.dram_tensor("ones_col", [128, 1], f32r, kind="ExternalInput").ap()
    ones_row = nc.dram_tensor("ones_row", [1, 128], f32r, kind="ExternalInput").ap()
    out_ext = nc.dram_tensor("out", [SPC, H], f32, kind="ExternalOutput").ap()

    with tile.TileContext(nc) as tc:
        with (
            tc.tile_pool(name="dram", bufs=1, space="DRAM") as dram_pool,
            tc.tile_pool(name="vstore", bufs=1) as v_store,
        ):
            qkrope = dram_pool.tile([2 * HPC, 128, S], f32r,
                                    name="qkrope")  # [q0..q3, k0..k3]
            partial = dram_pool.tile([S, H], f32, name="partial")
            rs_out = dram_pool.tile([SPC, H], f32, name="rs_out")

            with tc.tile_pool(name="wqk", bufs=1) as wqk_pool:
                wq_sb = wqk_pool.tile([128, KT, DPC], bf16, tag="wq")
                wk_sb = wqk_pool.tile([128, KT, DPC], bf16, tag="wk")

                # ====== Phase V: V projection (natural [s, d] layout) ========
                v_tiles = []
                with (
                    tc.tile_pool(name="wv", bufs=1) as wv_pool,
                    tc.tile_pool(name="xt2", bufs=3) as xt2_pool,
                    tc.tile_pool(name="psB", bufs=2, space="PSUM") as psB,
                ):
                    wv_sb = wv_pool.tile([128, KT, DPC], bf16, tag="wv")
                    for g in range(4):  # split so first matmuls start early
                        nc.sync.dma_start(
                            wv_sb[:, 8 * g:8 * (g + 1), :],
                            wv.rearrange("k p n -> p k n")[:, 8 * g:8 * (g + 1), :],
                        )
                    # prefetch Wq/Wk on the scalar-engine HWDGE queue
                    for g in range(4):
                        nc.scalar.dma_start(
                            wq_sb[:, :, 128 * g:128 * (g + 1)],
                            wq.rearrange("k p n -> p k n")[:, :, 128 * g:128 * (g + 1)],
                        )
                        nc.scalar.dma_start(
                            wk_sb[:, :, 128 * g:128 * (g + 1)],
                            wk.rearrange("k p n -> p k n")[:, :, 128 * g:128 * (g + 1)],
                        )
                    for st in range(ST):
                        x_sb = xt2_pool.tile([128, KT, 128], bf16, tag="x2")
                        nc.sync.dma_start(
                            x_sb[:],
                            xt.rearrange("k p s -> p k s")[:, :,
                                                           st * 128:(st + 1) * 128],
                        )
                        ps = psB.tile([128, DPC], f32, tag="vp")
                        for kt in range(KT):
                            nc.tensor.matmul(
                                ps[:], x_sb[:, kt, :], wv_sb[:, kt, :],
                                start=(kt == 0), stop=(kt == KT - 1),
                            )
                        v_t = v_store.tile([128, DPC], f32r, tag=f"v{st}",
                                           name=f"v{st}")
                        nc.scalar.copy(v_t[:], ps[:])
                        v_tiles.append(v_t)

                # ====== Phase QK: Q/K projections + RoPE (spill to DRAM) =====
                with (
                    tc.tile_pool(name="xt1", bufs=2) as xt1_pool,
                    tc.tile_pool(name="ropec", bufs=2) as rope_pool,
                    tc.tile_pool(name="rmp", bufs=1) as rm_pool,
                    tc.tile_pool(name="qktmp", bufs=2) as qktmp_pool,
                    tc.tile_pool(name="psA", bufs=2, space="PSUM") as psA,
                ):
                    rm_sb = rm_pool.tile([128, 128], f32r, tag="rm")
                    nc.sync.dma_start(rm_sb[:], rmat[:])
                    for sc in range(NSC):
                        s0 = sc * SCH
                        x_sb = xt1_pool.tile([128, KT, SCH], bf16, tag="x")
                        nsplit = 4 if sc == 0 else 1
                        for g in range(nsplit):
                            kspan = KT // nsplit
                            nc.sync.dma_start(
                                x_sb[:, kspan * g:kspan * (g + 1), :],
                                xt.rearrange("k p s -> p k s")[
                                    :, kspan * g:kspan * (g + 1), s0:s0 + SCH],
                            )
                        cos_sb = rope_pool.tile([128, SCH], f32, tag="cos")
                        sin_sb = rope_pool.tile([128, SCH], f32, tag="sin")
                        nc.scalar.dma_start(cos_sb[:], cost[:, s0:s0 + SCH])
                        nc.scalar.dma_start(sin_sb[:], sint[:, s0:s0 + SCH])
                        for pi, w_sb in ((0, wq_sb), (1, wk_sb)):
                            for h in range(HPC):
                                ps = psA.tile([128, SCH], f32, tag="proj")
                                for kt in range(KT):
                                    nc.tensor.matmul(
                                        ps[:],
                                        w_sb[:, kt, h * 128:(h + 1) * 128],
                                        x_sb[:, kt, :],
                                        start=(kt == 0),
                                        stop=(kt == KT - 1),
                                    )
                                raw = qktmp_pool.tile([128, SCH], f32r, tag="raw")
                                nc.scalar.copy(raw[:], ps[:])
                                psr = psA.tile([128, SCH], f32, tag="rot")
                                nc.tensor.matmul(psr[:], rm_sb[:], raw[:],
                                                 start=True, stop=True)
                                t1 = qktmp_pool.tile([128, SCH], f32, tag="t1")
                                nc.vector.tensor_mul(t1[:], raw[:], cos_sb[:])
                                t2 = qktmp_pool.tile([128, SCH], f32, tag="t2")
                                nc.vector.tensor_mul(t2[:], psr[:], sin_sb[:])
                                rope_t = qktmp_pool.tile([128, SCH], f32r,
                                                         tag="rope")
                                nc.vector.tensor_add(rope_t[:], t1[:], t2[:])
                                nc.sync.dma_start(
                                    qkrope[pi * HPC + h, :, s0:s0 + SCH],
                                    rope_t[:],
                                )

            # ====== Phase attn+o_proj: fused, qc-outer ======================
            with (
                tc.tile_pool(name="wo", bufs=1) as wo_pool,
                tc.tile_pool(name="ksb", bufs=2) as k_pool,
                tc.tile_pool(name="qsb", bufs=2) as q_pool,
                tc.tile_pool(name="pt", bufs=8) as pt_pool,
                tc.tile_pool(name="tmp", bufs=1) as tmp_pool,
                tc.tile_pool(name="attnmisc", bufs=3) as misc_pool,
                tc.tile_pool(name="otp", bufs=2) as ot_pool,
                tc.tile_pool(name="drain", bufs=4) as drain_pool,
                tc.tile_pool(name="psC", bufs=1, space="PSUM") as psC,
            ):
                wo_sb = wo_pool.tile([128, HPC, H], f32r, tag="wo")
                for g in range(4):
                    nc.scalar.dma_start(
                        wo_sb[:, g, :],
                        wo.rearrange("h p n -> p h n")[:, g, :],
                    )
                oc_sb = misc_pool.tile([128, 1], f32r, tag="ones_c", bufs=1)
                or_sb = misc_pool.tile([1, 128], f32r, tag="ones_r", bufs=1)
                nc.sync.dma_start(oc_sb[:], ones_col[:])
                nc.sync.dma_start(or_sb[:], ones_row[:])

                for qc in range(NQC):
                    q0 = qc * QCH
                    ot_cur = []
                    for h in range(HPC):
                        k_sb = k_pool.tile([128, S], f32r, tag="k")
                        nc.sync.dma_start(k_sb[:], qkrope[HPC + h])
                        q_sb = q_pool.tile([128, QCH], f32r, tag="q")
                        nc.sync.dma_start(q_sb[:], qkrope[h, :, q0:q0 + QCH])

                        # scores^T + exp, interleaved with attn@V accumulation
                        ps_o = psC.tile([128, QCH], f32, tag="vmm", bufs=2)
                        pts = []
                        for kt in range(NKT):
                            ps_s = psC.tile([128, QCH], f32, tag="scores",
                                            bufs=3)
                            nc.tensor.matmul(
                                ps_s[:],
                                k_sb[:, kt * 128:(kt + 1) * 128],
                                q_sb[:],
                                start=True, stop=True,
                            )
                            pt = pt_pool.tile([128, QCH], f32r, tag="pt")
                            nc.scalar.activation(
                                pt[:], ps_s[:],
                                mybir.ActivationFunctionType.Exp, scale=SCALE,
                            )
                            pts.append(pt)
                            if kt >= 2:
                                kv = kt - 2
                                nc.tensor.matmul(
                                    ps_o[:],
                                    v_tiles[kv][:, h * 128:(h + 1) * 128],
                                    pts[kv][:],
                                    start=(kv == 0), stop=False,
                                )
                        for kv in (NKT - 2, NKT - 1):
                            nc.tensor.matmul(
                                ps_o[:],
                                v_tiles[kv][:, h * 128:(h + 1) * 128],
                                pts[kv][:],
                                start=False, stop=(kv == NKT - 1),
                            )

                        # denominator: batched tree sum of the 16 P^T tiles
                        tmp = tmp_pool.tile([128, 8, QCH], f32, tag="tr")
                        for i in range(8):
                            nc.vector.tensor_add(tmp[:, i, :],
                                                 pts[2 * i][:], pts[2 * i + 1][:])
                        nc.vector.tensor_add(tmp[:, 0:4, :],
                                             tmp[:, 0:4, :], tmp[:, 4:8, :])
                        nc.vector.tensor_add(tmp[:, 0:2, :],
                                             tmp[:, 0:2, :], tmp[:, 2:4, :])
                        t_sum = misc_pool.tile([128, QCH], f32r, tag="tsum",
                                               bufs=2)
                        nc.vector.tensor_add(t_sum[:], tmp[:, 0, :], tmp[:, 1, :])

                        # cross-partition sum -> broadcast -> reciprocal
                        ps_sum = psC.tile([1, QCH], f32, tag="sumbc", bufs=1)
                        nc.tensor.matmul(ps_sum[:], oc_sb[:], t_sum[:],
                                         start=True, stop=True)
                        sum_sb = misc_pool.tile([1, QCH], f32r, tag="sum_sb")
                        nc.vector.tensor_copy(sum_sb[:], ps_sum[:])
                        ps_bc = psC.tile([128, QCH], f32, tag="sumbc", bufs=1)
                        nc.tensor.matmul(ps_bc[:], or_sb[:], sum_sb[:],
                                         start=True, stop=True)
                        recip_sb = misc_pool.tile([128, QCH], f32, tag="recip")
                        nc.vector.reciprocal(recip_sb[:], ps_bc[:])

                        ot_t = ot_pool.tile([128, QCH], f32r, tag=f"ot{h}",
                                            name=f"ot{h}")
                        nc.vector.tensor_mul(ot_t[:], ps_o[:], recip_sb[:])
                        ot_cur.append(ot_t)

                    # fused o_proj for this q-chunk
                    for qt_local in range(QCH // 128):
                        qt = qc * (QCH // 128) + qt_local
                        for nci in range(H // 512):
                            n0 = nci * 512
                            ps = psC.tile([128, 512], f32, tag="op", bufs=2)
                            for h in range(HPC):
                                nc.tensor.matmul(
                                    ps[:],
                                    ot_cur[h][:,
                                              qt_local * 128:(qt_local + 1) * 128],
                                    wo_sb[:, h, n0:n0 + 512],
                                    start=(h == 0), stop=(h == HPC - 1),
                                )
                            dr = drain_pool.tile([128, 512], f32, tag="dr")
                            nc.vector.tensor_copy(dr[:], ps[:])
                            nc.gpsimd.dma_start(
                                partial[qt * 128:(qt + 1) * 128, n0:n0 + 512],
                                dr[:],
                            )

                    # chunked ReduceScatter over this q-chunk's rows
                    if with_collective and RS_CHUNKS == NQC:
                        nc.gpsimd.collective_compute(
                            "ReduceScatter",
                            mybir.AluOpType.add,
                            replica_groups=[list(range(NC))],
                            ins=[partial[q0:q0 + QCH, :].opt()],
                            outs=[rs_out[qc * (QCH // NC):
                                         (qc + 1) * (QCH // NC), :].opt()],
                        )

            # ====== Final: (single RS) + output ==============================
            if with_collective and RS_CHUNKS != NQC:
                nc.gpsimd.collective_compute(
                    "ReduceScatter",
                    mybir.AluOpType.add,
                    replica_groups=[list(range(NC))],
                    ins=[partial.opt()],
                    outs=[rs_out.opt()],
                )
            if with_collective:
                nc.gpsimd.dma_start(out_ext[:], rs_out[:])
            else:
                nc.gpsimd.dma_start(out_ext[:], partial[:SPC, :])

    nc.compile()
    return nc


def _host_prep(positions, hidden_states, Wq, Wk, Wv, Wo):
    X = np.asarray(hidden_states, dtype=np.float32).reshape(S, H)
    XT = np.ascontiguousarray(X.T).astype(bf16_np).reshape(KT, 128, S)

    pos = np.asarray(positions).astype(np.float32)
    inv_freq = (1.0 / (ROPE_THETA ** (np.arange(0, D, 2, dtype=np.float32) / D)))
    freqs = pos[:, None] * inv_freq[None, :]
    emb = np.concatenate([freqs, freqs], axis=-1)        # [S, D]
    cosT = np.ascontiguousarray(np.cos(emb).astype(np.float32).T)  # [128, S]
    sinT = np.ascontiguousarray(np.sin(emb).astype(np.float32).T)

    rm = np.zeros((128, 128), np.float32)
    idx = np.arange(64)
    rm[64 + idx, idx] = -1.0   # out[0:64]  = -in[64:128]
    rm[idx, 64 + idx] = 1.0    # out[64:128] = in[0:64]

    Wq = np.asarray(Wq, dtype=np.float32)
    Wk = np.asarray(Wk, dtype=np.float32)
    Wv = np.asarray(Wv, dtype=np.float32)
    Wo = np.asarray(Wo, dtype=np.float32)

    in_maps = []
    for c in range(NC):
        sl = slice(DPC * c, DPC * (c + 1))
        wq_c = np.ascontiguousarray(Wq[sl, :].T).astype(bf16_np).reshape(KT, 128, DPC)
        wk_c = np.ascontiguousarray(Wk[sl, :].T).astype(bf16_np).reshape(KT, 128, DPC)
        wv_c = np.ascontiguousarray(Wv[sl, :].T).astype(bf16_np).reshape(KT, 128, DPC)
        wo_c = np.ascontiguousarray(Wo[:, sl].T).reshape(HPC, 128, H)
        in_maps.append({
            "xt": XT, "wq": wq_c, "wk": wk_c, "wv": wv_c, "wo": wo_c,
            "cost": cosT, "sint": sinT, "rmat": rm,
            "ones_col": np.ones((128, 1), np.float32),
            "ones_row": np.ones((1, 128), np.float32),
        })
    return in_maps


def _assemble(results):
    """Reassemble full [1, S, H] output from per-core RS slices."""
    if RS_CHUNKS == NQC:
        # core c, chunk qc holds global rows qc*QCH + c*(QCH//NC) + r
        full = np.empty((NQC, NC, QCH // NC, H), np.float32)
        for c in range(NC):
            full[:, c] = results[c]["out"].reshape(NQC, QCH // NC, H)
        return full.reshape(1, S, H)
    out = np.concatenate([results[c]["out"] for c in range(NC)], axis=0)
    return out.reshape(1, S, H)


def kernel(positions, hidden_states, Wq, Wk, Wv, Wo):
    if "nc" not in _CACHE:
        _CACHE["nc"] = _build()
    nc = _CACHE["nc"]
    in_maps = _host_prep(positions, hidden_states, Wq, Wk, Wv, Wo)
    res = run_bass_kernel_spmd(nc, in_maps, list(range(NC)))
    return _assemble(res.results).astype(np.float32)



# revision 3
# speedup vs baseline: 5048.7360x; 5048.7360x over previous
"""Self-contained TP-over-heads DeepseekAttention kernel for 8 TRN2 cores, v2.

Sharding: tensor-parallel across heads (4 heads/core). Per core: Q/K/V
projections (bf16 matmuls), RoPE (f32 math, bf16 store), q/k kept SBUF-
resident, attention with transposed-scores layout (bf16 matmuls, f32 psum),
row-parallel o_proj fused per q-chunk (bf16), chunked bf16 ReduceScatter.
Host reassembles the 8 per-core [256, 4096] bf16 slices and upcasts.
"""

import numpy as np
import ml_dtypes

import concourse.bass as bass
from concourse import bass_isa
import concourse.mybir as mybir
import concourse.tile as tile
from concourse import bacc
from concourse.bass_utils import run_bass_kernel_spmd

# problem shapes (hardcoded per contract)
S = 2048
H = 4096
NH = 32
D = 128
NC = 8
HPC = NH // NC          # 4 heads per core
DPC = HPC * D           # 512 head-dims per core
KT = H // 128           # 32 contraction tiles over hidden
SCH = 512               # s-chunk for projections
NSC = S // SCH          # 4
ST = S // 128           # 16 s-tiles
QCH = 512               # q-chunk in attention
NQC = S // QCH          # 4
NKT = S // 128          # 16 k-tiles in attention
SPC = S // NC           # 256 rows of output per core

f32 = mybir.dt.float32
f32r = mybir.dt.float32r
bf16 = mybir.dt.bfloat16
bf16_np = ml_dtypes.bfloat16

ROPE_THETA = 10000.0
SCALE = float(1.0 / np.sqrt(D))

_CACHE: dict = {}

# ReduceScatter chunk plan: (emit_at_qc, row_start, row_end); None = at end.
# _assemble must mirror this row mapping.
RS_PLAN = ((2, 0, 1024), (None, 1024, 2048))


def _build(with_collective=True, reps=1, rs_plan=RS_PLAN):
    nc = bacc.Bacc("TRN2", target_bir_lowering=False, debug=False, num_devices=NC)

    # ---- I/O ----
    xt = nc.dram_tensor("xt", [KT, 128, S], bf16, kind="ExternalInput").ap()
    wq = nc.dram_tensor("wq", [KT, 128, DPC], bf16, kind="ExternalInput").ap()
    wk = nc.dram_tensor("wk", [KT, 128, DPC], bf16, kind="ExternalInput").ap()
    wv = nc.dram_tensor("wv", [KT, 128, DPC], bf16, kind="ExternalInput").ap()
    wo = nc.dram_tensor("wo", [HPC, 128, H], bf16, kind="ExternalInput").ap()
    cost = nc.dram_tensor("cost", [128, S], f32, kind="ExternalInput").ap()
    sint = nc.dram_tensor("sint", [128, S], f32, kind="ExternalInput").ap()
    rmat = nc.dram_tensor("rmat", [128, 128], f32r, kind="ExternalInput").ap()
    out_ext = nc.dram_tensor("out", [SPC, H], bf16, kind="ExternalOutput").ap()

    with tile.TileContext(nc) as tc:
        with tc.tile_pool(name="dram", bufs=1, space="DRAM") as dram_pool:
            partial = dram_pool.tile([S, H], bf16, name="partial")
            rs_out = dram_pool.tile([SPC, H], bf16, name="rs_out")

            for rep in range(reps):
                _emit_rep(nc, tc, rep, with_collective,
                          xt, wq, wk, wv, wo, cost, sint, rmat,
                          out_ext, partial, rs_out, rs_plan)

    nc.compile()
    return nc


def _emit_rep(nc, tc, rep, with_collective,
              xt, wq, wk, wv, wo, cost, sint, rmat,
              out_ext, partial, rs_out, rs_plan=RS_PLAN):
    r = f"r{rep}"
    with (
        tc.tile_pool(name=f"store{r}", bufs=1) as store,
        tc.tile_pool(name=f"wqk{r}", bufs=1) as wqk_pool,
    ):
        # persistent SBUF tiles: V (bf16) and rope'd Q/K (bf16), full seq
        v_tiles = [store.tile([128, DPC], bf16, tag=f"v{st}", name=f"v{st}{r}")
                   for st in range(ST)]
        q_tiles = [store.tile([128, S], bf16, tag=f"q{h}", name=f"q{h}{r}")
                   for h in range(HPC)]
        k_tiles = [store.tile([128, S], bf16, tag=f"k{h}", name=f"k{h}{r}")
                   for h in range(HPC)]
        qk_tiles = (q_tiles, k_tiles)

        wq_sb = wqk_pool.tile([128, KT, DPC], bf16, tag="wq")
        wk_sb = wqk_pool.tile([128, KT, DPC], bf16, tag="wk")

        # ====== Phase V: V projection ([s, d] layout) ====================
        # kt-group-major over blocks of 4 s-tiles: the PE can start on wv
        # group 0 as soon as it lands instead of stalling on the full wv
        # load, and 4 in-flight psum banks decouple the 4 s-tiles.
        with (
            tc.tile_pool(name=f"wv{r}", bufs=1) as wv_pool,
            tc.tile_pool(name=f"xt2{r}", bufs=6) as xt2_pool,
            tc.tile_pool(name=f"psB{r}", bufs=1, space="PSUM") as psB,
        ):
            wv_sb = wv_pool.tile([128, KT, DPC], bf16, tag="wv")

            def load_x(st):
                x_sb = xt2_pool.tile([128, KT, 128], bf16, tag="x2")
                nc.sync.dma_start(
                    x_sb[:],
                    xt.rearrange("k p s -> p k s")[:, :,
                                                   st * 128:(st + 1) * 128])
                return x_sb

            # first block: interleave half-x-tiles with 4-kt wv groups so
            # the PE's first matmuls start after ~1MB instead of ~6MB
            x_tiles = []
            for st in range(4):
                x_sb = xt2_pool.tile([128, KT, 128], bf16, tag="x2",
                                     name=f"x2f{st}{r}")
                for half in (0, 1):
                    nc.sync.dma_start(
                        x_sb[:, 16 * half:16 * (half + 1), :],
                        xt.rearrange("k p s -> p k s")[
                            :, 16 * half:16 * (half + 1),
                            st * 128:(st + 1) * 128])
                    g4 = 2 * st + half
                    nc.sync.dma_start(
                        wv_sb[:, 4 * g4:4 * (g4 + 1), :],
                        wv.rearrange("k p n -> p k n")[:, 4 * g4:4 * (g4 + 1), :])
                x_tiles.append(x_sb)

            for blk in range(4):
                if blk > 0:
                    x_tiles = [load_x(4 * blk + i) for i in range(4)]
                # prefetch Wq/Wk during blocks 1-2 (needed at QK start)
                if blk in (1, 2):
                    for g in (2 * (blk - 1), 2 * blk - 1):
                        nc.scalar.dma_start(
                            wq_sb[:, 8 * g:8 * (g + 1), :],
                            wq.rearrange("k p n -> p k n")[:, 8 * g:8 * (g + 1), :])
                        nc.scalar.dma_start(
                            wk_sb[:, 8 * g:8 * (g + 1), :],
                            wk.rearrange("k p n -> p k n")[:, 8 * g:8 * (g + 1), :])
                ps4 = [psB.tile([128, DPC], f32, tag=f"vp{i}",
                                name=f"vps{i}b{blk}{r}") for i in range(4)]
                for g in range(4):
                    for i in range(4):
                        for kt in range(8 * g, 8 * g + 8):
                            nc.tensor.matmul(
                                ps4[i][:], x_tiles[i][:, kt, :], wv_sb[:, kt, :],
                                start=(kt == 0), stop=(kt == KT - 1),
                            )
                for i in range(4):
                    nc.scalar.copy(v_tiles[4 * blk + i][:], ps4[i][:])

        # ====== Phase QK: Q/K projections + RoPE -> SBUF bf16 ============
        with (
            tc.tile_pool(name=f"xt1{r}", bufs=2) as xt1_pool,
            tc.tile_pool(name=f"ropec{r}", bufs=1) as rope_pool,
            tc.tile_pool(name=f"rmp{r}", bufs=1) as rm_pool,
            tc.tile_pool(name=f"qktmp{r}", bufs=1) as qktmp_pool,
            tc.tile_pool(name=f"psA{r}", bufs=1, space="PSUM") as psA,
        ):
            rm_sb = rm_pool.tile([128, 128], f32r, tag="rm")
            nc.sync.dma_start(rm_sb[:], rmat[:])
            pending = None
            for sc in range(NSC):
                s0 = sc * SCH
                x_sb = xt1_pool.tile([128, KT, SCH], bf16, tag="x")
                nsplit = 4
                for g in range(nsplit):
                    kspan = KT // nsplit
                    nc.sync.dma_start(
                        x_sb[:, kspan * g:kspan * (g + 1), :],
                        xt.rearrange("k p s -> p k s")[
                            :, kspan * g:kspan * (g + 1), s0:s0 + SCH],
                    )
                cos_sb = rope_pool.tile([128, SCH], f32, tag="cos")
                sin_sb = rope_pool.tile([128, SCH], f32, tag="sin")
                nc.scalar.dma_start(cos_sb[:], cost[:, s0:s0 + SCH])
                nc.scalar.dma_start(sin_sb[:], sint[:, s0:s0 + SCH])
                # software-pipelined: the rotate matmul + rope vector ops of
                # group n are emitted after group n+1's projection matmuls so
                # the PE never waits on the Act-engine psum->raw copy.
                def emit_rope(pi, h, raw, cos_c, sin_c, sc0):
                    psr = psA.tile([128, SCH], f32, tag="rot", bufs=2)
                    nc.tensor.matmul(psr[:], rm_sb[:], raw[:],
                                     start=True, stop=True)
                    t1 = qktmp_pool.tile([128, SCH], f32, tag="t1", bufs=2)
                    nc.vector.tensor_mul(t1[:], raw[:], cos_c[:])
                    t2 = qktmp_pool.tile([128, SCH], f32, tag="t2", bufs=2)
                    nc.vector.tensor_mul(t2[:], psr[:], sin_c[:])
                    nc.vector.tensor_add(
                        qk_tiles[pi][h][:, sc0:sc0 + SCH], t1[:], t2[:])

                for pi, w_sb in ((0, wq_sb), (1, wk_sb)):
                    for h in range(HPC):
                        ps = psA.tile([128, SCH], f32, tag="proj", bufs=2)
                        for kt in range(KT):
                            nc.tensor.matmul(
                                ps[:],
                                w_sb[:, kt, h * 128:(h + 1) * 128],
                                x_sb[:, kt, :],
                                start=(kt == 0),
                                stop=(kt == KT - 1),
                            )
                        raw = qktmp_pool.tile([128, SCH], f32r, tag="raw",
                                              bufs=2)
                        nc.scalar.copy(raw[:], ps[:])
                        if pending is not None:
                            emit_rope(*pending)
                        pending = (pi, h, raw, cos_sb, sin_sb, s0)
            if pending is not None:
                emit_rope(*pending)

        # ====== Phase attn + o_proj: fused, qc-outer ======================
        with (
            tc.tile_pool(name=f"wo{r}", bufs=1) as wo_pool,
            tc.tile_pool(name=f"pt{r}", bufs=8) as pt_pool,
            tc.tile_pool(name=f"tmp{r}", bufs=1) as tmp_pool,
            tc.tile_pool(name=f"attnmisc{r}", bufs=2) as misc_pool,
            tc.tile_pool(name=f"otp{r}", bufs=2) as ot_pool,
            tc.tile_pool(name=f"drain{r}", bufs=2) as drain_pool,
            tc.tile_pool(name=f"psC{r}", bufs=1, space="PSUM") as psC,
            tc.tile_pool(name=f"psO{r}", bufs=1, space="PSUM") as psO,
        ):
            wo_sb = wo_pool.tile([128, HPC, H], bf16, tag="wo")
            for g in range(4):
                nc.scalar.dma_start(
                    wo_sb[:, g, :],
                    wo.rearrange("h p n -> p h n")[:, g, :],
                )

            NPAIR = NKT // 2   # 8 double-width score tiles per head

            def emit_rs(r0, r1):
                # Emitted only once the drains for rows [r0, r1) are already
                # complete, so the SEQ-side semaphore wait never blocks the
                # Pool queue (which also carries partition_all_reduce ops).
                nc.gpsimd.collective_compute(
                    "ReduceScatter",
                    mybir.AluOpType.add,
                    replica_groups=[list(range(NC))],
                    ins=[partial[r0:r1, :].opt()],
                    outs=[rs_out[r0 // NC:r1 // NC, :].opt()],
                )

            for qc in range(NQC):
                q0 = qc * QCH
                if with_collective:
                    for at_qc, r0, r1 in rs_plan:
                        if at_qc == qc:
                            emit_rs(r0, r1)
                ot_cur = []
                for h in range(HPC):
                    k_sb = k_tiles[h]
                    q_sb = q_tiles[h]

                    # scores^T (pairs of k-tiles in one 2-bank psum tile),
                    # one 1024-wide exp per pair, interleaved with attn@V
                    ps_o = psC.tile([128, QCH], f32, tag="vmm", bufs=2)
                    pts = []

                    def attn_v(kv):
                        nc.tensor.matmul(
                            ps_o[:],
                            v_tiles[kv][:, h * 128:(h + 1) * 128],
                            pts[kv // 2][:, (kv % 2) * QCH:(kv % 2 + 1) * QCH],
                            start=(kv == 0), stop=(kv == NKT - 1),
                        )

                    for j in range(NPAIR):
                        ps_s = psC.tile([128, 2 * QCH], f32, tag="scores",
                                        bufs=2)
                        for half in (0, 1):
                            kt = 2 * j + half
                            nc.tensor.matmul(
                                ps_s[:, half * QCH:(half + 1) * QCH],
                                k_sb[:, kt * 128:(kt + 1) * 128],
                                q_sb[:, q0:q0 + QCH],
                                start=True, stop=True,
                            )
                        pt = pt_pool.tile([128, 2 * QCH], bf16, tag="pt")
                        nc.scalar.activation(
                            pt[:], ps_s[:],
                            mybir.ActivationFunctionType.Exp, scale=SCALE,
                        )
                        pts.append(pt)
                        if j >= 2:   # 2-pair lag so attn@V never waits on exp
                            attn_v(2 * (j - 2))
                            attn_v(2 * (j - 2) + 1)
                    for kv in range(NKT - 4, NKT):
                        attn_v(kv)

                    # denominator: tree sum (DVE) -> partition all-reduce
                    # (gpsimd) -> reciprocal + scale (DVE); no PE involved
                    tmp = tmp_pool.tile([128, 4, 2 * QCH], f32, tag="tr")
                    for i in range(4):
                        nc.vector.tensor_add(tmp[:, i, :],
                                             pts[2 * i][:], pts[2 * i + 1][:])
                    nc.vector.tensor_add(tmp[:, 0:2, :],
                                         tmp[:, 0:2, :], tmp[:, 2:4, :])
                    nc.vector.tensor_add(tmp[:, 0, :],
                                         tmp[:, 0, :], tmp[:, 1, :])
                    t_sum = misc_pool.tile([128, QCH], f32, tag="tsum",
                                           bufs=2)
                    nc.vector.tensor_add(t_sum[:], tmp[:, 0, 0:QCH],
                                         tmp[:, 0, QCH:2 * QCH])
                    den_bc = misc_pool.tile([128, QCH], f32, tag="denbc",
                                            bufs=2)
                    nc.gpsimd.partition_all_reduce(
                        den_bc[:], t_sum[:], 128, bass_isa.ReduceOp.add)
                    recip_sb = misc_pool.tile([128, QCH], f32, tag="recip")
                    nc.vector.reciprocal(recip_sb[:], den_bc[:])

                    ot_t = ot_pool.tile([128, QCH], bf16, tag=f"ot{h}",
                                        name=f"ot{h}{r}")
                    nc.vector.tensor_mul(ot_t[:], ps_o[:], recip_sb[:])
                    ot_cur.append(ot_t)

                # fused o_proj for this q-chunk; 1024-wide drains.
                # bf16 ReduceScatter chunks are emitted as soon as their rows
                # are drained; the final chunk is only 128 rows so the exposed
                # collective tail is small.
                for qt_local in range(QCH // 128):
                    qt = qc * (QCH // 128) + qt_local
                    for p in range(H // 1024):
                        n0 = p * 1024
                        ps_a = psO.tile([128, 512], f32, tag="op", bufs=2)
                        ps_b = psO.tile([128, 512], f32, tag="op", bufs=2)
                        for h in range(HPC):
                            st_ap = ot_cur[h][:,
                                              qt_local * 128:(qt_local + 1) * 128]
                            nc.tensor.matmul(
                                ps_a[:], st_ap, wo_sb[:, h, n0:n0 + 512],
                                start=(h == 0), stop=(h == HPC - 1),
                            )
                            nc.tensor.matmul(
                                ps_b[:], st_ap, wo_sb[:, h, n0 + 512:n0 + 1024],
                                start=(h == 0), stop=(h == HPC - 1),
                            )
                        dr = drain_pool.tile([128, 1024], bf16, tag="dr")
                        nc.scalar.copy(dr[:, 0:512], ps_a[:])
                        nc.vector.tensor_copy(dr[:, 512:1024], ps_b[:])
                        nc.sync.dma_start(
                            partial[qt * 128:(qt + 1) * 128, n0:n0 + 1024],
                            dr[:],
                        )

            if with_collective:
                for at_qc, r0, r1 in rs_plan:
                    if at_qc is None:
                        emit_rs(r0, r1)
                # rows for RS0-2 are long done -> this copy overlaps RS3;
                # scalar queue is idle here and nothing latency-critical
                # queues behind it
                nc.scalar.dma_start(out_ext[:3 * QCH // NC, :],
                                    rs_out[:3 * QCH // NC, :])
                nc.scalar.dma_start(out_ext[3 * QCH // NC:, :],
                                    rs_out[3 * QCH // NC:, :])
            else:
                nc.gpsimd.dma_start(out_ext[:], partial[:SPC, :])


def _host_prep(positions, hidden_states, Wq, Wk, Wv, Wo):
    X = np.asarray(hidden_states, dtype=np.float32).reshape(S, H)
    XT = np.ascontiguousarray(X.T).astype(bf16_np).reshape(KT, 128, S)

    pos = np.asarray(positions).astype(np.float32)
    inv_freq = (1.0 / (ROPE_THETA ** (np.arange(0, D, 2, dtype=np.float32) / D)))
    freqs = pos[:, None] * inv_freq[None, :]
    emb = np.concatenate([freqs, freqs], axis=-1)        # [S, D]
    cosT = np.ascontiguousarray(np.cos(emb).astype(np.float32).T)  # [128, S]
    sinT = np.ascontiguousarray(np.sin(emb).astype(np.float32).T)

    rm = np.zeros((128, 128), np.float32)
    idx = np.arange(64)
    rm[64 + idx, idx] = -1.0   # out[0:64]  = -in[64:128]
    rm[idx, 64 + idx] = 1.0    # out[64:128] = in[0:64]

    Wq = np.asarray(Wq, dtype=np.float32)
    Wk = np.asarray(Wk, dtype=np.float32)
    Wv = np.asarray(Wv, dtype=np.float32)
    Wo = np.asarray(Wo, dtype=np.float32)

    in_maps = []
    for c in range(NC):
        sl = slice(DPC * c, DPC * (c + 1))
        wq_c = np.ascontiguousarray(Wq[sl, :].T).astype(bf16_np).reshape(KT, 128, DPC)
        wk_c = np.ascontiguousarray(Wk[sl, :].T).astype(bf16_np).reshape(KT, 128, DPC)
        wv_c = np.ascontiguousarray(Wv[sl, :].T).astype(bf16_np).reshape(KT, 128, DPC)
        wo_c = np.ascontiguousarray(Wo[:, sl].T).astype(bf16_np).reshape(HPC, 128, H)
        in_maps.append({
            "xt": XT, "wq": wq_c, "wk": wk_c, "wv": wv_c, "wo": wo_c,
            "cost": cosT, "sint": sinT, "rmat": rm,
        })
    return in_maps


def _assemble(results):
    """Reassemble full [1, S, H] output from per-core RS slices.

    For each RS chunk [r0, r1), core c's rs_out rows [r0//NC, r1//NC) hold
    global rows [r0 + c*L, r0 + (c+1)*L) where L = (r1 - r0) // NC.
    """
    full = np.empty((S, H), np.float32)
    for c in range(NC):
        out_c = results[c]["out"].astype(np.float32)
        for _, r0, r1 in RS_PLAN:
            L = (r1 - r0) // NC
            full[r0 + c * L:r0 + (c + 1) * L] = out_c[r0 // NC:r0 // NC + L]
    return full.reshape(1, S, H)


def kernel(positions, hidden_states, Wq, Wk, Wv, Wo):
    if "nc" not in _CACHE:
        _CACHE["nc"] = _build()
    nc = _CACHE["nc"]
    in_maps = _host_prep(positions, hidden_states, Wq, Wk, Wv, Wo)
    res = run_bass_kernel_spmd(nc, in_maps, list(range(NC)))
    return _assemble(res.results)


# revision 6
# speedup vs baseline: 6049.7832x; 1.1983x over previous
"""Self-contained TP-over-heads DeepseekAttention kernel for 8 TRN2 cores, v2.

Sharding: tensor-parallel across heads (4 heads/core). Per core: Q/K/V
projections (bf16 matmuls), RoPE (f32 math, bf16 store), q/k kept SBUF-
resident, attention with transposed-scores layout (bf16 matmuls, f32 psum),
row-parallel o_proj fused per q-chunk (bf16), chunked bf16 ReduceScatter.
Host reassembles the 8 per-core [256, 4096] bf16 slices and upcasts.
"""

import numpy as np
import ml_dtypes

import concourse.bass as bass
from concourse import bass_isa
import concourse.mybir as mybir
import concourse.tile as tile
from concourse import bacc
from concourse.bass_utils import run_bass_kernel_spmd

# problem shapes (hardcoded per contract)
S = 2048
H = 4096
NH = 32
D = 128
NC = 8
HPC = NH // NC          # 4 heads per core
DPC = HPC * D           # 512 head-dims per core
KT = H // 128           # 32 contraction tiles over hidden
SCH = 512               # s-chunk for projections
NSC = S // SCH          # 4
ST = S // 128           # 16 s-tiles
QCH = 512               # q-chunk in attention
NQC = S // QCH          # 4
NKT = S // 128          # 16 k-tiles in attention
SPC = S // NC           # 256 rows of output per core

f32 = mybir.dt.float32
f32r = mybir.dt.float32r
bf16 = mybir.dt.bfloat16
bf16_np = ml_dtypes.bfloat16

ROPE_THETA = 10000.0
SCALE = float(1.0 / np.sqrt(D))

_CACHE: dict = {}

# ReduceScatter chunk plan: (emit_at_qc, row_start, row_end); None = at end.
# _assemble must mirror this row mapping.
RS_PLAN = ((3, 0, 1536), (None, 1536, 2048))


def _build(with_collective=True, reps=1, rs_plan=RS_PLAN):
    nc = bacc.Bacc("TRN2", target_bir_lowering=False, debug=False, num_devices=NC)

    # ---- I/O ----
    xt = nc.dram_tensor("xt", [KT, 128, S], bf16, kind="ExternalInput").ap()
    wq = nc.dram_tensor("wq", [KT, 128, DPC], bf16, kind="ExternalInput").ap()
    wk = nc.dram_tensor("wk", [KT, 128, DPC], bf16, kind="ExternalInput").ap()
    wv = nc.dram_tensor("wv", [KT, 128, DPC], bf16, kind="ExternalInput").ap()
    wo = nc.dram_tensor("wo", [HPC, 128, H], bf16, kind="ExternalInput").ap()
    cost = nc.dram_tensor("cost", [128, S], f32, kind="ExternalInput").ap()
    sint = nc.dram_tensor("sint", [128, S], f32, kind="ExternalInput").ap()
    rmat = nc.dram_tensor("rmat", [128, 128], f32r, kind="ExternalInput").ap()
    out_ext = nc.dram_tensor("out", [SPC, H], bf16, kind="ExternalOutput").ap()

    with tile.TileContext(nc) as tc:
        with tc.tile_pool(name="dram", bufs=1, space="DRAM") as dram_pool:
            partial = dram_pool.tile([S, H], bf16, name="partial")
            rs_out = dram_pool.tile([SPC, H], bf16, name="rs_out")

            for rep in range(reps):
                _emit_rep(nc, tc, rep, with_collective,
                          xt, wq, wk, wv, wo, cost, sint, rmat,
                          out_ext, partial, rs_out, rs_plan)

    nc.compile()
    return nc


def _emit_rep(nc, tc, rep, with_collective,
              xt, wq, wk, wv, wo, cost, sint, rmat,
              out_ext, partial, rs_out, rs_plan=RS_PLAN):
    r = f"r{rep}"
    with (
        tc.tile_pool(name=f"store{r}", bufs=1) as store,
        tc.tile_pool(name=f"wqk{r}", bufs=1) as wqk_pool,
    ):
        # persistent SBUF tiles: V (bf16) and rope'd Q/K (bf16), full seq
        v_tiles = [store.tile([128, DPC], bf16, tag=f"v{st}", name=f"v{st}{r}")
                   for st in range(ST)]
        q_tiles = [store.tile([128, S], bf16, tag=f"q{h}", name=f"q{h}{r}")
                   for h in range(HPC)]
        k_tiles = [store.tile([128, S], bf16, tag=f"k{h}", name=f"k{h}{r}")
                   for h in range(HPC)]
        qk_tiles = (q_tiles, k_tiles)

        wq_sb = wqk_pool.tile([128, KT, DPC], bf16, tag="wq")
        wk_sb = wqk_pool.tile([128, KT, DPC], bf16, tag="wk")

        # ====== Phase V: V projection ([s, d] layout) ====================
        # kt-group-major over blocks of 4 s-tiles: the PE can start on wv
        # group 0 as soon as it lands instead of stalling on the full wv
        # load, and 4 in-flight psum banks decouple the 4 s-tiles.
        with (
            tc.tile_pool(name=f"wv{r}", bufs=1) as wv_pool,
            tc.tile_pool(name=f"xt2{r}", bufs=6) as xt2_pool,
            tc.tile_pool(name=f"psB{r}", bufs=1, space="PSUM") as psB,
        ):
            wv_sb = wv_pool.tile([128, KT, DPC], bf16, tag="wv")

            def load_x(st):
                x_sb = xt2_pool.tile([128, KT, 128], bf16, tag="x2")
                nc.sync.dma_start(
                    x_sb[:],
                    xt.rearrange("k p s -> p k s")[:, :,
                                                   st * 128:(st + 1) * 128])
                return x_sb

            # first block: interleave half-x-tiles with 4-kt wv groups so
            # the PE's first matmuls start after ~1MB instead of ~6MB
            x_tiles = []
            for st in range(4):
                x_sb = xt2_pool.tile([128, KT, 128], bf16, tag="x2",
                                     name=f"x2f{st}{r}")
                for half in (0, 1):
                    nc.sync.dma_start(
                        x_sb[:, 16 * half:16 * (half + 1), :],
                        xt.rearrange("k p s -> p k s")[
                            :, 16 * half:16 * (half + 1),
                            st * 128:(st + 1) * 128])
                    g4 = 2 * st + half
                    nc.sync.dma_start(
                        wv_sb[:, 4 * g4:4 * (g4 + 1), :],
                        wv.rearrange("k p n -> p k n")[:, 4 * g4:4 * (g4 + 1), :])
                x_tiles.append(x_sb)

            for blk in range(4):
                if blk > 0:
                    x_tiles = [load_x(4 * blk + i) for i in range(4)]
                # prefetch Wq/Wk during blocks 1-2 (needed at QK start)
                if blk in (1, 2):
                    for g in (2 * (blk - 1), 2 * blk - 1):
                        nc.scalar.dma_start(
                            wq_sb[:, 8 * g:8 * (g + 1), :],
                            wq.rearrange("k p n -> p k n")[:, 8 * g:8 * (g + 1), :])
                        nc.scalar.dma_start(
                            wk_sb[:, 8 * g:8 * (g + 1), :],
                            wk.rearrange("k p n -> p k n")[:, 8 * g:8 * (g + 1), :])
                ps4 = [psB.tile([128, DPC], f32, tag=f"vp{i}",
                                name=f"vps{i}b{blk}{r}") for i in range(4)]
                for g in range(4):
                    for i in range(4):
                        for kt in range(8 * g, 8 * g + 8):
                            nc.tensor.matmul(
                                ps4[i][:], x_tiles[i][:, kt, :], wv_sb[:, kt, :],
                                start=(kt == 0), stop=(kt == KT - 1),
                            )
                for i in range(4):
                    nc.scalar.copy(v_tiles[4 * blk + i][:], ps4[i][:])

        # ====== Phase QK: Q/K projections + RoPE -> SBUF bf16 ============
        with (
            tc.tile_pool(name=f"xt1{r}", bufs=2) as xt1_pool,
            tc.tile_pool(name=f"ropec{r}", bufs=1) as rope_pool,
            tc.tile_pool(name=f"rmp{r}", bufs=1) as rm_pool,
            tc.tile_pool(name=f"qktmp{r}", bufs=1) as qktmp_pool,
            tc.tile_pool(name=f"psA{r}", bufs=1, space="PSUM") as psA,
        ):
            rm_sb = rm_pool.tile([128, 128], f32r, tag="rm")
            nc.sync.dma_start(rm_sb[:], rmat[:])
            pending = None
            for sc in range(NSC):
                s0 = sc * SCH
                x_sb = xt1_pool.tile([128, KT, SCH], bf16, tag="x")
                nsplit = 4
                for g in range(nsplit):
                    kspan = KT // nsplit
                    nc.sync.dma_start(
                        x_sb[:, kspan * g:kspan * (g + 1), :],
                        xt.rearrange("k p s -> p k s")[
                            :, kspan * g:kspan * (g + 1), s0:s0 + SCH],
                    )
                cos_sb = rope_pool.tile([128, SCH], f32, tag="cos")
                sin_sb = rope_pool.tile([128, SCH], f32, tag="sin")
                nc.scalar.dma_start(cos_sb[:], cost[:, s0:s0 + SCH])
                nc.scalar.dma_start(sin_sb[:], sint[:, s0:s0 + SCH])
                # software-pipelined: the rotate matmul + rope vector ops of
                # group n are emitted after group n+1's projection matmuls so
                # the PE never waits on the Act-engine psum->raw copy.
                def emit_rope(pi, h, raw, cos_c, sin_c, sc0):
                    psr = psA.tile([128, SCH], f32, tag="rot", bufs=2)
                    nc.tensor.matmul(psr[:], rm_sb[:], raw[:],
                                     start=True, stop=True)
                    t1 = qktmp_pool.tile([128, SCH], f32, tag="t1", bufs=2)
                    nc.vector.tensor_mul(t1[:], raw[:], cos_c[:])
                    t2 = qktmp_pool.tile([128, SCH], f32, tag="t2", bufs=2)
                    nc.vector.tensor_mul(t2[:], psr[:], sin_c[:])
                    nc.vector.tensor_add(
                        qk_tiles[pi][h][:, sc0:sc0 + SCH], t1[:], t2[:])

                for pi, w_sb in ((0, wq_sb), (1, wk_sb)):
                    for h in range(HPC):
                        ps = psA.tile([128, SCH], f32, tag="proj", bufs=2)
                        for kt in range(KT):
                            nc.tensor.matmul(
                                ps[:],
                                w_sb[:, kt, h * 128:(h + 1) * 128],
                                x_sb[:, kt, :],
                                start=(kt == 0),
                                stop=(kt == KT - 1),
                            )
                        raw = qktmp_pool.tile([128, SCH], f32r, tag="raw",
                                              bufs=2)
                        nc.scalar.copy(raw[:], ps[:])
                        if pending is not None:
                            emit_rope(*pending)
                        pending = (pi, h, raw, cos_sb, sin_sb, s0)
            if pending is not None:
                emit_rope(*pending)

        # ====== Phase attn + o_proj: fused, qc-outer ======================
        with (
            tc.tile_pool(name=f"wo{r}", bufs=1) as wo_pool,
            tc.tile_pool(name=f"pt{r}", bufs=8) as pt_pool,
            tc.tile_pool(name=f"tmp{r}", bufs=1) as tmp_pool,
            tc.tile_pool(name=f"attnmisc{r}", bufs=2) as misc_pool,
            tc.tile_pool(name=f"otp{r}", bufs=2) as ot_pool,
            tc.tile_pool(name=f"drain{r}", bufs=2) as drain_pool,
            tc.tile_pool(name=f"psC{r}", bufs=1, space="PSUM") as psC,
            tc.tile_pool(name=f"psO{r}", bufs=1, space="PSUM") as psO,
        ):
            wo_sb = wo_pool.tile([128, HPC, H], bf16, tag="wo")
            for g in range(4):
                nc.scalar.dma_start(
                    wo_sb[:, g, :],
                    wo.rearrange("h p n -> p h n")[:, g, :],
                )

            NPAIR = NKT // 2   # 8 double-width score tiles per head

            def emit_rs(r0, r1):
                # Emitted only once the drains for rows [r0, r1) are already
                # complete, so the SEQ-side semaphore wait never blocks the
                # Pool queue (which also carries partition_all_reduce ops).
                nc.gpsimd.collective_compute(
                    "ReduceScatter",
                    mybir.AluOpType.add,
                    replica_groups=[list(range(NC))],
                    ins=[partial[r0:r1, :].opt()],
                    outs=[rs_out[r0 // NC:r1 // NC, :].opt()],
                )

            for qc in range(NQC):
                q0 = qc * QCH
                if with_collective:
                    for at_qc, r0, r1 in rs_plan:
                        if at_qc == qc:
                            emit_rs(r0, r1)
                ot_cur = []
                for h in range(HPC):
                    k_sb = k_tiles[h]
                    q_sb = q_tiles[h]

                    # scores^T (pairs of k-tiles in one 2-bank psum tile),
                    # one 1024-wide exp per pair, interleaved with attn@V
                    ps_o = psC.tile([128, QCH], f32, tag="vmm", bufs=2)
                    pts = []

                    def attn_v(kv):
                        nc.tensor.matmul(
                            ps_o[:],
                            v_tiles[kv][:, h * 128:(h + 1) * 128],
                            pts[kv // 2][:, (kv % 2) * QCH:(kv % 2 + 1) * QCH],
                            start=(kv == 0), stop=(kv == NKT - 1),
                        )

                    for j in range(NPAIR):
                        ps_s = psC.tile([128, 2 * QCH], f32, tag="scores",
                                        bufs=2)
                        for half in (0, 1):
                            kt = 2 * j + half
                            nc.tensor.matmul(
                                ps_s[:, half * QCH:(half + 1) * QCH],
                                k_sb[:, kt * 128:(kt + 1) * 128],
                                q_sb[:, q0:q0 + QCH],
                                start=True, stop=True,
                            )
                        pt = pt_pool.tile([128, 2 * QCH], bf16, tag="pt")
                        nc.scalar.activation(
                            pt[:], ps_s[:],
                            mybir.ActivationFunctionType.Exp, scale=SCALE,
                        )
                        pts.append(pt)
                        if j >= 2:   # 2-pair lag so attn@V never waits on exp
                            attn_v(2 * (j - 2))
                            attn_v(2 * (j - 2) + 1)
                    for kv in range(NKT - 4, NKT):
                        attn_v(kv)

                    # denominator: tree sum (DVE) -> partition all-reduce
                    # (gpsimd) -> reciprocal + scale (DVE); no PE involved
                    tmp = tmp_pool.tile([128, 4, 2 * QCH], f32, tag="tr")
                    for i in range(4):
                        nc.vector.tensor_add(tmp[:, i, :],
                                             pts[2 * i][:], pts[2 * i + 1][:])
                    nc.vector.tensor_add(tmp[:, 0:2, :],
                                         tmp[:, 0:2, :], tmp[:, 2:4, :])
                    nc.vector.tensor_add(tmp[:, 0, :],
                                         tmp[:, 0, :], tmp[:, 1, :])
                    t_sum = misc_pool.tile([128, QCH], f32, tag="tsum",
                                           bufs=2)
                    nc.vector.tensor_add(t_sum[:], tmp[:, 0, 0:QCH],
                                         tmp[:, 0, QCH:2 * QCH])
                    den_bc = misc_pool.tile([128, QCH], f32, tag="denbc",
                                            bufs=2)
                    nc.gpsimd.partition_all_reduce(
                        den_bc[:], t_sum[:], 128, bass_isa.ReduceOp.add)
                    recip_sb = misc_pool.tile([128, QCH], f32, tag="recip")
                    nc.vector.reciprocal(recip_sb[:], den_bc[:])

                    ot_t = ot_pool.tile([128, QCH], bf16, tag=f"ot{h}",
                                        name=f"ot{h}{r}")
                    nc.vector.tensor_mul(ot_t[:], ps_o[:], recip_sb[:])
                    ot_cur.append(ot_t)

                # fused o_proj for this q-chunk; 1024-wide drains.
                # bf16 ReduceScatter chunks are emitted as soon as their rows
                # are drained; the final chunk is only 128 rows so the exposed
                # collective tail is small.
                for qt_local in range(QCH // 128):
                    qt = qc * (QCH // 128) + qt_local
                    for p in range(H // 1024):
                        n0 = p * 1024
                        ps_a = psO.tile([128, 512], f32, tag="op", bufs=2)
                        ps_b = psO.tile([128, 512], f32, tag="op", bufs=2)
                        for h in range(HPC):
                            st_ap = ot_cur[h][:,
                                              qt_local * 128:(qt_local + 1) * 128]
                            nc.tensor.matmul(
                                ps_a[:], st_ap, wo_sb[:, h, n0:n0 + 512],
                                start=(h == 0), stop=(h == HPC - 1),
                            )
                            nc.tensor.matmul(
                                ps_b[:], st_ap, wo_sb[:, h, n0 + 512:n0 + 1024],
                                start=(h == 0), stop=(h == HPC - 1),
                            )
                        dr = drain_pool.tile([128, 1024], bf16, tag="dr")
                        nc.scalar.copy(dr[:, 0:512], ps_a[:])
                        nc.vector.tensor_copy(dr[:, 512:1024], ps_b[:])
                        nc.sync.dma_start(
                            partial[qt * 128:(qt + 1) * 128, n0:n0 + 1024],
                            dr[:],
                        )

            if with_collective:
                for at_qc, r0, r1 in rs_plan:
                    if at_qc is None:
                        emit_rs(r0, r1)
                # rows for the first RS chunk are long done -> this copy
                # overlaps the final RS; scalar queue is idle here
                nc.scalar.dma_start(out_ext[:3 * QCH // NC, :],
                                    rs_out[:3 * QCH // NC, :])
                nc.scalar.dma_start(out_ext[3 * QCH // NC:, :],
                                    rs_out[3 * QCH // NC:, :])
            else:
                nc.gpsimd.dma_start(out_ext[:], partial[:SPC, :])


def _host_prep(positions, hidden_states, Wq, Wk, Wv, Wo):
    X = np.asarray(hidden_states, dtype=np.float32).reshape(S, H)
    XT = np.ascontiguousarray(X.T).astype(bf16_np).reshape(KT, 128, S)

    pos = np.asarray(positions).astype(np.float32)
    inv_freq = (1.0 / (ROPE_THETA ** (np.arange(0, D, 2, dtype=np.float32) / D)))
    freqs = pos[:, None] * inv_freq[None, :]
    emb = np.concatenate([freqs, freqs], axis=-1)        # [S, D]
    cosT = np.ascontiguousarray(np.cos(emb).astype(np.float32).T)  # [128, S]
    sinT = np.ascontiguousarray(np.sin(emb).astype(np.float32).T)

    rm = np.zeros((128, 128), np.float32)
    idx = np.arange(64)
    rm[64 + idx, idx] = -1.0   # out[0:64]  = -in[64:128]
    rm[idx, 64 + idx] = 1.0    # out[64:128] = in[0:64]

    Wq = np.asarray(Wq, dtype=np.float32)
    Wk = np.asarray(Wk, dtype=np.float32)
    Wv = np.asarray(Wv, dtype=np.float32)
    Wo = np.asarray(Wo, dtype=np.float32)

    in_maps = []
    for c in range(NC):
        sl = slice(DPC * c, DPC * (c + 1))
        wq_c = np.ascontiguousarray(Wq[sl, :].T).astype(bf16_np).reshape(KT, 128, DPC)
        wk_c = np.ascontiguousarray(Wk[sl, :].T).astype(bf16_np).reshape(KT, 128, DPC)
        wv_c = np.ascontiguousarray(Wv[sl, :].T).astype(bf16_np).reshape(KT, 128, DPC)
        wo_c = np.ascontiguousarray(Wo[:, sl].T).astype(bf16_np).reshape(HPC, 128, H)
        in_maps.append({
            "xt": XT, "wq": wq_c, "wk": wk_c, "wv": wv_c, "wo": wo_c,
            "cost": cosT, "sint": sinT, "rmat": rm,
        })
    return in_maps


def _assemble(results):
    """Reassemble full [1, S, H] output from per-core RS slices.

    For each RS chunk [r0, r1), core c's rs_out rows [r0//NC, r1//NC) hold
    global rows [r0 + c*L, r0 + (c+1)*L) where L = (r1 - r0) // NC.
    """
    full = np.empty((S, H), np.float32)
    for c in range(NC):
        out_c = results[c]["out"].astype(np.float32)
        for _, r0, r1 in RS_PLAN:
            L = (r1 - r0) // NC
            full[r0 + c * L:r0 + (c + 1) * L] = out_c[r0 // NC:r0 // NC + L]
    return full.reshape(1, S, H)


def kernel(positions, hidden_states, Wq, Wk, Wv, Wo):
    if "nc" not in _CACHE:
        _CACHE["nc"] = _build()
    nc = _CACHE["nc"]
    in_maps = _host_prep(positions, hidden_states, Wq, Wk, Wv, Wo)
    res = run_bass_kernel_spmd(nc, in_maps, list(range(NC)))
    return _assemble(res.results)
